# revision 16
# baseline (speedup 1.0000x reference)
"""Causal multi-head attention (QKV proj + 16-head causal attention) on 8 TRN2 cores.

Problem: x [4, 2048, 1024], W [3072, 1024], b [3072] -> out [4, 2048, 1024].
H=16 heads, D=64. Sharding: core c = (batch b = c // 2, head-group g = c % 2);
each core computes batch b, heads g*8 .. g*8+8, producing out[b][:, g*512:(g+1)*512].
No cross-core communication needed.

Device kernel (per core), all matmuls bf16 with f32 PSUM accumulation.
ScalarE (exp over ~18M live logits) and TensorE are co-bottlenecks; the
whole kernel is a single software pipeline paced by them:
  - Attention is processed per (tq-chunk J of 512, head pair hp) as one
    MERGED loop over tk-tiles i: S^T pair matmuls (head dim on 64 partitions,
    both heads issued adjacently for concurrent sub-array execution) -> exp
    on ScalarE (2-slot PSUM groups, trimmed to causally-live columns) ->
    128x128 upper-triangular mask multiply on GpSimd (diagonal tiles only)
    -> v-stationary P@v pair accumulating y^T[65, tq] (row 64 = softmax
    denominator via a prefilled ones column in vA).  S^T stays two i-tiles
    ahead of P@v, so ScalarE paces the loop and TensorE fills its slack.
  - Phase tails (PSUM->SBUF casts, PE transposes back to token-major,
    denominator reciprocals, normalization) are deferred until after the
    NEXT phase's first two S^T pairs, so exp never waits on tail work.
    Both heads' y^T pack into one [128, 512] tile via partition-shifted DVE
    casts -> 4 PE transposes/pair; denominators of a head-pair PAIR collect
    on partitions {0,32,64,96} of a [97, 512] tile -> 4 [97,128] transposes
    + reciprocals per two phases.
  - QKV projection (chunked by 512 tokens) fills between-phase slots: Q/K
    bias folds into the PSUM->SBUF cast (per-partition tensor_scalar_add),
    V bias via one rank-1 matmul per t-tile, V copied with one strided CAST
    per t-tile into vA.  ~36 identity transposes at t=0 warm the PE clock
    (HAM) before the first real matmul.
"""

import numpy as np
import ml_dtypes

B, T, C = 4, 2048, 1024
H, D = 16, 64
HPC = 8            # heads per core
OC = HPC * D       # 512 output cols per core
NCORES = 8

_cache = {}


def _build_bass():
    import concourse.mybir as mybir
    import concourse.tile as tile
    from concourse import bacc
    from concourse.masks import make_identity, make_upper_triangular

    f32 = mybir.dt.float32
    bf16 = mybir.dt.bfloat16
    EXP = mybir.ActivationFunctionType.Exp

    nc = bacc.Bacc(None)
    xt_d = nc.declare_dram_parameter("xt", [C, T], bf16, isOutput=False)
    wt_d = nc.declare_dram_parameter("wt", [C, 3 * OC], bf16, isOutput=False)
    bcc_d = nc.declare_dram_parameter("bcc", [128, 8], f32, isOutput=False)
    btr_d = nc.declare_dram_parameter("btr", [1, OC], bf16, isOutput=False)
    out_d = nc.declare_dram_parameter("out", [T, OC], f32, isOutput=True)

    CT = C // 128     # 8 c-tiles
    TT = T // 128     # 16 t-tiles
    TJ = T // 512     # 4 big t-chunks

    with tile.TileContext(nc) as tc:
        with (
            tc.tile_pool(name="persist", bufs=1) as persist,
            tc.tile_pool(name="xtp", bufs=2) as xtp,
            tc.tile_pool(name="qtp", bufs=2) as qtp,
            tc.tile_pool(name="ptp", bufs=18) as ptp,
            tc.tile_pool(name="ytp", bufs=2) as ytp,
            tc.tile_pool(name="denp", bufs=2) as denp,
            tc.tile_pool(name="rcp", bufs=2) as rcp,
            tc.tile_pool(name="osbp", bufs=4) as osbp,
            tc.tile_pool(name="spsum", bufs=2, space="PSUM") as spsum,
            tc.tile_pool(name="psvp", bufs=2, space="PSUM") as psvp,
            tc.tile_pool(name="workp", bufs=1, space="PSUM") as workp,
            tc.tile_pool(name="tpsum", bufs=1, space="PSUM") as tpsum,
        ):
            # ---- persistent SBUF tensors ----
            wt = persist.tile([128, CT, 3 * OC], bf16)     # [c%128, c//128, o]
            kT = persist.tile([128, OC // 128, T], bf16)   # [o%128, o//128, t]
            vA = persist.tile([128, TT, HPC, D + 1], bf16)  # v + ones col
            bcc = persist.tile([128, 8], f32)              # Q/K bias, col=o-tile
            btr = persist.tile([1, OC], bf16)              # V bias row
            ones = persist.tile([1, 128], bf16)
            ut = persist.tile([128, 128], bf16)            # upper-tri (incl diag)
            iden = persist.tile([128, 128], bf16)

            nc.gpsimd.memset(ones[:, :], 1.0)
            nc.gpsimd.memset(vA[:], 1.0)                   # pre-fill ones column
            make_identity(nc, iden[:, :])
            make_upper_triangular(nc, ut[:, :], val=1.0, diag=True)

            # PE clock (HAM) warmup: ~36 transposes of the identity keep the
            # PE busy from ~1us so the first real matmuls run at 2.4 GHz
            warm = tpsum.tile([128, 4, 256], bf16, name="warm", tag="tps")
            for k in range(36):
                nc.tensor.transpose(
                    warm[:, k % 4, 0:128], iden[:, :], iden[:, :])

            # early DMAs: first Q/K weight block, tokens chunk 0, Q/K bias
            # (keep [512-row] -> [128, 4, *] DMA shapes: this split maps
            # src row = ct*128 + p; other shapes pair dims differently)
            nc.sync.dma_start(wt[:, 0:4, 0:256], wt_d[0:512, 0:256])
            nc.sync.dma_start(wt[:, 4:8, 0:256], wt_d[512:1024, 0:256])
            xts = [None] * TJ
            qts = [None] * TJ

            def load_chunk(tj):
                xts[tj] = xtp.tile([128, CT, 512], bf16, name=f"xt{tj}", tag="xt")
                nc.sync.dma_start(xts[tj][:, 0:4, :],
                                  xt_d[0:512, tj * 512:(tj + 1) * 512])
                nc.sync.dma_start(xts[tj][:, 4:8, :],
                                  xt_d[512:1024, tj * 512:(tj + 1) * 512])
                qts[tj] = qtp.tile([128, 4, 512], bf16, name=f"qt{tj}", tag="qt")

            load_chunk(0)
            nc.sync.dma_start(bcc[:, :], bcc_d[:, :])

            def qk_od(tj, g):
                """Project q and k o-tile g for token chunk tj."""
                xtt, qtt = xts[tj], qts[tj]
                for which in range(2):                     # 0 = q, 1 = k
                    ps = psvp.tile([128, 512], f32, name="ps", tag="psv")
                    w0 = g * 256 + which * 128
                    for ci in range(CT):
                        nc.tensor.matmul(
                            ps[:, :],
                            lhsT=wt[:, ci, w0:w0 + 128],
                            rhs=xtt[:, ci, :],
                            start=(ci == 0), stop=(ci == CT - 1))
                    if which == 0:
                        nc.vector.tensor_scalar_add(
                            qtt[:, g, :], ps[:, :], bcc[:, 2 * g:2 * g + 1])
                    else:
                        nc.vector.tensor_scalar_add(
                            kT[:, g, tj * 512:(tj + 1) * 512], ps[:, :],
                            bcc[:, 2 * g + 1:2 * g + 2])

            def v_tl(tj, tl):
                """Project v for one 128-token tile (runs inside phase bodies)."""
                xtt = xts[tj]
                tt = tj * 4 + tl
                ps = workp.tile([128, 512], f32, name="vps", tag="wk")
                for ci in range(CT):
                    nc.tensor.matmul(
                        ps[:, :],
                        lhsT=xtt[:, ci, tl * 128:(tl + 1) * 128],
                        rhs=wt[:, ci, 2 * OC:3 * OC],
                        start=(ci == 0), stop=False)
                nc.tensor.matmul(
                    ps[:, :], lhsT=ones[:, :], rhs=btr[:, :],
                    start=False, stop=True)
                nc.vector.tensor_copy(vA[:, tt, :, 0:D], ps[:, :])

            # ---- merged attention phase machinery ----
            st = {}                                        # per-phase state

            def s_pair(J, hp, i):
                """One S^T pair + exp + mask for tk-tile i."""
                c0 = max(0, (i - 4 * J) * 128)
                qtt = qts[J]
                ptt = ptp.tile([128, 2, 512], bf16, name="pt", tag="pt")
                sp = spsum.tile([128, 2, 512], f32, name="sp", tag="sp")
                for hc in range(2):
                    kp = hc * 64
                    nc.tensor.matmul(
                        sp[:, hc, c0:512],
                        lhsT=kT[kp:kp + 64, hp, i * 128:(i + 1) * 128],
                        rhs=qtt[kp:kp + 64, hp, c0:512],
                        start=True, stop=True)
                nc.scalar.activation(
                    ptt[:, 0:2, c0:512], sp[:, 0:2, c0:512], EXP, scale=0.125)
                if i >= 4 * J:                             # diagonal tile
                    for hc in range(2):
                        nc.gpsimd.tensor_mul(
                            ptt[:, hc, c0:c0 + 128],
                            ptt[:, hc, c0:c0 + 128], ut[:, :])
                return ptt

            def pv_mm(J, hp, i, ptt, psvs, ni):
                for hc in range(2):
                    c0 = max(0, (i - 4 * J) * 128)
                    nc.tensor.matmul(
                        psvs[hc][0:65, c0:512],
                        lhsT=vA[:, i, 2 * hp + hc, :],
                        rhs=ptt[:, hc, c0:512],
                        start=(i == 0), stop=(i == ni - 1),
                        skip_group_check=(c0 > 0))

            def phase_head(J, hp):
                st[(J, hp)] = {"pts": [s_pair(J, hp, 0), s_pair(J, hp, 1)]}

            def phase_body(J, hp, extra=None):
                """P@v interleaved with S^T two tiles ahead; `extra` emits
                one unit of between-work (e.g. a V tile) per i step."""
                ni = 4 * J + 4
                ss = st[(J, hp)]
                psvs = [psvp.tile([128, 512], f32, name="psv", tag="psv")
                        for _ in range(2)]
                ss["psvs"] = psvs
                pts = ss["pts"]
                for i in range(ni):
                    if extra:
                        extra(i)
                    pv_mm(J, hp, i, pts[i], psvs, ni)
                    if i + 2 < ni:
                        pts.append(s_pair(J, hp, i + 2))

            osbs = {}

            def phase_tail(J, hp):
                ss = st.pop((J, hp))
                psvs = ss["psvs"]
                if hp == 0:
                    osbs[J] = [osbp.tile([128, OC], f32, name=f"osb{J}_{jl}",
                                         tag=f"osb{jl}") for jl in range(4)]
                if hp % 2 == 0:
                    # head-pair-PAIR shared tiles (this hp and the next)
                    ss2 = st.setdefault((J, hp + 1), {})
                    dn2 = denp.tile([97, 512], bf16, name="dn", tag="dn")
                    nc.gpsimd.memset(dn2[:, :], 0.0)
                    tps = tpsum.tile([128, 4, 256], bf16, name="tps", tag="tps")
                    ss2["dn2"], ss2["tps"] = dn2, tps
                else:
                    dn2, tps = ss["dn2"], ss["tps"]
                half = (hp % 2) * 128
                ytpair = ytp.tile([128, 512], bf16, name="yt", tag="yt")
                for hc in range(2):
                    nc.vector.tensor_copy(
                        ytpair[hc * 64:(hc + 1) * 64, :], psvs[hc][0:64, :])
                    r = 32 * (2 * (hp % 2) + hc)
                    nc.vector.tensor_copy(dn2[r:r + 1, :], psvs[hc][64:65, :])
                for jl in range(4):
                    nc.tensor.transpose(
                        tps[:, jl, half:half + 128],
                        ytpair[:, jl * 128:(jl + 1) * 128], iden[:, :])
                if hp % 2 == 1:
                    st.pop((J, hp), None)
                    dtp = workp.tile([128, 4, 100], bf16, name="dtp", tag="wk")
                    for jl in range(4):
                        nc.tensor.transpose(
                            dtp[:, jl, 0:97],
                            dn2[:, jl * 128:(jl + 1) * 128], iden[0:97, 0:97])
                    rc = rcp.tile([128, 4, 4], f32, name="rc", tag="rc")
                    for r4 in range(4):
                        nc.vector.reciprocal(
                            rc[:, r4, :], dtp[:, :, 32 * r4:32 * r4 + 1])
                    for hq in (hp - 1, hp):
                        hf = (hq % 2) * 128
                        for jl in range(4):
                            for hc in range(2):
                                nc.vector.tensor_scalar_mul(
                                    osbs[J][jl][:, hq * 128 + hc * 64:
                                                hq * 128 + (hc + 1) * 64],
                                    tps[:, jl, hf + hc * 64:hf + (hc + 1) * 64],
                                    rc[:, 2 * (hq % 2) + hc, jl:jl + 1])
                if hp == 3:
                    for jl in range(4):
                        r0 = (4 * J + jl) * 128
                        nc.sync.dma_start(out_d[r0:r0 + 128, :],
                                          osbs[J][jl][:, :])
                    del osbs[J]

            # ---- emission schedule ----
            # Global phase order (J, hp); each iteration: this phase's first
            # two S^T pairs, then the previous phase's tail, then between-slot
            # QKV work for the NEXT phase, then this phase's merged body.
            qk_od(0, 0)
            # remaining weights (v first: needed by phase (0,0)'s body)
            nc.sync.dma_start(wt[:, 0:4, 2 * OC:3 * OC],
                              wt_d[0:512, 2 * OC:3 * OC])
            nc.sync.dma_start(wt[:, 4:8, 2 * OC:3 * OC],
                              wt_d[512:1024, 2 * OC:3 * OC])
            nc.sync.dma_start(btr[:, :], btr_d[:, :])
            for g in range(1, 4):
                nc.sync.dma_start(wt[:, 0:4, g * 256:(g + 1) * 256],
                                  wt_d[0:512, g * 256:(g + 1) * 256])
                nc.sync.dma_start(wt[:, 4:8, g * 256:(g + 1) * 256],
                                  wt_d[512:1024, g * 256:(g + 1) * 256])

            phases = [(J, hp) for J in range(TJ) for hp in range(4)]
            prev = None
            for idx, (J, hp) in enumerate(phases):
                phase_head(J, hp)
                if prev is not None:
                    phase_tail(*prev)
                # between-slot: QKV work feeding the upcoming phases
                if J == 0 and hp < 3:
                    qk_od(0, hp + 1)                       # rest of chunk 0
                if hp == 0 and J + 1 < TJ:
                    load_chunk(J + 1)
                if J + 1 < TJ:
                    qk_od(J + 1, hp)                       # chunk J+1, one od
                # body; phase (0,0) interleaves chunk-0 V tiles into its slack
                if J == 0 and hp == 0:
                    phase_body(J, hp, extra=lambda i: v_tl(0, i))
                    v_tl(1, 0)                             # chunk 1's first tile
                else:
                    if J + 1 < TJ:
                        v_tl(J + 1, hp)                    # chunk J+1, one tile
                    phase_body(J, hp)
                prev = (J, hp)
            phase_tail(*prev)

    nc.finalize()
    return nc


def _prep_inputs(x, W, b):
    """Build per-core input maps (host-side sharding + layout prep)."""
    in_maps = []
    for core in range(NCORES):
        bi, g = core // 2, core % 2
        h0 = g * HPC
        # weight rows, interleaved [q0,k0,q1,k1,q2,k2,q3,k3,v] by 128-row
        # o-tiles (o-tile g covers heads h0+2g, h0+2g+1)
        blocks = []
        for gg in range(4):
            r = (h0 + 2 * gg) * D
            blocks.append(np.arange(r, r + 128))           # q o-tile gg
            blocks.append(np.arange(C + r, C + r + 128))   # k o-tile gg
        blocks.append(np.arange(2 * C + h0 * D, 2 * C + h0 * D + OC))  # v
        rows = np.concatenate(blocks)
        Wc = W[rows, :]                                    # [1536, 1024]
        bcc = np.empty((128, 8), dtype=np.float32)
        for gg in range(4):
            r = (h0 + 2 * gg) * D
            bcc[:, 2 * gg] = b[r:r + 128]
            bcc[:, 2 * gg + 1] = b[C + r:C + r + 128]
        btr = b[2 * C + h0 * D:2 * C + h0 * D + OC]
        in_maps.append({
            "xt": np.ascontiguousarray(x[bi].T).astype(ml_dtypes.bfloat16),
            "wt": np.ascontiguousarray(Wc.T).astype(ml_dtypes.bfloat16),
            "bcc": bcc,
            "btr": btr.reshape(1, -1).astype(ml_dtypes.bfloat16),
        })
    return in_maps


def kernel(x, W, b):
    from concourse.bass_utils import run_bass_kernel_spmd

    if "nc" not in _cache:
        _cache["nc"] = _build_bass()
    nc = _cache["nc"]
    in_maps = _prep_inputs(np.asarray(x), np.asarray(W), np.asarray(b))
    res = run_bass_kernel_spmd(nc, in_maps, core_ids=list(range(NCORES)))
    out = np.empty((B, T, C), dtype=np.float32)
    for core in range(NCORES):
        bi, g = core // 2, core % 2
        out[bi][:, g * OC:(g + 1) * OC] = res.results[core]["out"]
    return out
